# revision 14
# baseline (speedup 1.0000x reference)
# Trainium2 Bass kernel for the ASE (axial squeeze attention) block.
#
# Sharding: pure data parallel over batch B=16 across 8 NeuronCores
# (2 batches per core); all params replicated.
#
# Math restructuring (host-side folds):
#  - BN scales fold into conv weights; biases applied during PSUM evictions.
#  - depthwise gate scale g folds into the q/k conv weights (diagonal
#    commutes); v keeps g on its ACT eviction since the raw v psum also
#    feeds the xx path.
#  - 1x1 convs commute with spatial means, so row/col attention only needs
#    the row/col sums of x (256x64 each), never full q/k maps.
#  - positional embeddings interpolated on host and folded into qk biases.
#  - softmax: exp on ACT; denominator via ones-matmul; normalization by
#    broadcasting the reciprocal row with a K=1 matmul.
#  - h_sigmoid(x)*gate = min(relu(x+3), 6) * (gate/6): the /6 folds into the
#    pointwise conv weights; the final conv runs in fp8e4 DoubleRow perf
#    mode (weights x64 to stay in fp8 normal range; /64 on the ACT evict).
#  - y stored bf16, upcast on host.
import numpy as np

import concourse.bass as bass
import concourse.mybir as mybir
import concourse.tile as tile
from concourse import bacc, bass_utils

F32 = mybir.dt.float32
F32R = mybir.dt.float32r
BF16 = mybir.dt.bfloat16
FP8 = mybir.dt.float8e4
AF = mybir.ActivationFunctionType
ALU = mybir.AluOpType
AX = mybir.AxisListType
DR = mybir.MatmulPerfMode.DoubleRow
USE_DR = True
SKIP_ATTN = False

B, DIM, H, W = 16, 256, 64, 64
KEY_DIM, HEADS = 16, 8
NH_KD = KEY_DIM * HEADS   # 128
DH = 2 * KEY_DIM * HEADS  # 256
POS = 16
N_CORES = 8
BPC = B // N_CORES        # batches per core

MMDT = F32R  # dtype of x / qkv weights feeding the PE

WBIG_COLS = 128 * 2 + 128 * 2 + 256 * 2 + 256 * 4 + 128   # q,k,v,pw,ident
WBIGB_COLS = 512 + 512 + 512 + 512 + 512 + 64   # wr,wc,wq,wk,wv,id64


def build_nc(bpc=BPC, h=H, w=W, chunk_h=8, num_devices=N_CORES, use_f32r=True,
             nrep=1, tiny_out=False):
    """Build the per-core Bass module."""
    global MMDT
    MMDT = F32R if use_f32r else F32
    hw = h * w
    n_chunks = h // chunk_h
    nc_cols = chunk_h * w      # spatial columns per chunk

    nc = bacc.Bacc("TRN2", target_bir_lowering=False, debug=False,
                   num_devices=num_devices)

    dram = {}

    def din(name, shape, dt=None):
        dram[name] = nc.dram_tensor(name, shape, dt or F32,
                                    kind="ExternalInput").ap()
        return dram[name]

    din("x", (bpc, DIM, hw), MMDT)
    din("wbig", (128, WBIG_COLS), MMDT)
    din("wbigb", (128, WBIGB_COLS), BF16)
    din("wp8", (128, 512), FP8)
    din("qkbias", (128, 512))
    din("params", (128, 20))
    din("onesW", (max(h, w), 1), BF16)
    din("ones1", (1, 64), BF16)
    y_cols = nc_cols if tiny_out else hw
    y_d = nc.dram_tensor("y", (bpc, DIM, y_cols), BF16,
                         kind="ExternalOutput").ap()

    with tile.TileContext(nc) as tc:
        _emit(nc, tc, dram, y_d, bpc, h, w, hw, chunk_h, n_chunks, nc_cols,
              nrep, tiny_out)
    nc.compile()
    return nc


def _emit(nc, tc, dram, y_d, bpc, h, w, hw, chunk_h, n_chunks, nc_cols,
          nrep=1, tiny_out=False):
    from contextlib import ExitStack
    with ExitStack() as _ctx:
        _emit_body(_ctx, nc, tc, dram, y_d, bpc, h, w, hw, chunk_h, n_chunks,
                   nc_cols, nrep, tiny_out)


def _emit_body(ctx, nc, tc, dram, y_d, bpc, h, w, hw, chunk_h, n_chunks,
               nc_cols, nrep=1, tiny_out=False):
    ctx.enter_context(nc.allow_low_precision(
        reason="bf16/fp8 matmul operand rounding"))
    # ---- persistent weights / params (loaded once) ----
    wp = ctx.enter_context(tc.tile_pool(name="weights", bufs=1))

    wbig = wp.tile([128, WBIG_COLS], MMDT, tag="wbig")
    nc.scalar.dma_start(out=wbig, in_=dram["wbig"])
    wbigb = wp.tile([128, WBIGB_COLS], BF16, tag="wbigb")
    nc.scalar.dma_start(out=wbigb, in_=dram["wbigb"])
    # DoubleRow stationary tiles must be native [p, 2, m] 3-D tiles
    wp8t = []
    for mo in range(2):
        t8 = wp.tile([128, 2, 128], FP8, tag=f"wp8_{mo}", name=f"wp8_{mo}")
        nc.scalar.dma_start(out=t8.rearrange("p i f -> p (i f)"),
                          in_=dram["wp8"][:, 256 * mo:256 * (mo + 1)])
        wp8t.append(t8)
    params = wp.tile([128, 20], F32, tag="params")
    nc.scalar.dma_start(out=params, in_=dram["params"])
    qkb = wp.tile([128, 512], F32, tag="qkb")
    nc.scalar.dma_start(out=qkb, in_=dram["qkbias"])

    def _slices(tile_, widths):
        out, off = [], 0
        for wd in widths:
            out.append(tile_[:, off:off + wd])
            off += wd
        return out

    (wqT0, wqT1, wkT0, wkT1, wvT0, wvT1, wpw0, wpw1, wpw2, wpw3,
     ident128) = _slices(
        wbig, [NH_KD, NH_KD, NH_KD, NH_KD, DH, DH,
               DIM, DIM, DIM, DIM, 128])
    wqT, wkT, wvT = [wqT0, wqT1], [wkT0, wkT1], [wvT0, wvT1]
    wpwT = [wpw0, wpw1, wpw2, wpw3]
    (wrT0, wrT1, wcT0, wcT1, wqTp0, wqTp1, wkTp0, wkTp1, wvTb0, wvTb1,
     ident64b) = _slices(
        wbigb, [DH, DH, DH, DH, 256, 256, 256, 256, DH, DH, 64])
    wrT, wcT = [wrT0, wrT1], [wcT0, wcT1]
    wqTp, wkTp, wvTb = [wqTp0, wqTp1], [wkTp0, wkTp1], [wvTb0, wvTb1]
    ident64 = ident64b[:64, :]
    onesW = wp.tile([max(h, w), 1], BF16, tag="onesW")   # value = W (mean fold)
    nc.scalar.dma_start(out=onesW, in_=dram["onesW"])
    ones1 = wp.tile([1, 64], BF16, tag="ones1")
    nc.scalar.dma_start(out=ones1, in_=dram["ones1"])

    # param columns
    zsv = [params[:, g:g + 1] for g in range(2)]          # zscale for v grps
    zbias = [params[:, 2 + g:3 + g] for g in range(4)]    # q, k, v0, v1
    bv_att = [params[:, 6 + m:7 + m] for m in range(2)]
    brv = [params[:, 8 + m:9 + m] for m in range(2)]
    bcc = [params[:, 10 + m:11 + m] for m in range(2)]
    bp3 = [params[:, 12 + m:13 + m] for m in range(2)]
    bpw6 = [params[:, 14 + m:15 + m] for m in range(2)]

    # ---- pools ----
    px = ctx.enter_context(tc.tile_pool(name="x", bufs=2))
    pa = ctx.enter_context(tc.tile_pool(name="attn", bufs=1))
    pz = ctx.enter_context(tc.tile_pool(name="z", bufs=5))
    pc = ctx.enter_context(tc.tile_pool(name="chunk", bufs=3))
    pout = ctx.enter_context(tc.tile_pool(name="outb", bufs=3))
    pp = ctx.enter_context(tc.tile_pool(name="psum", bufs=1, space="PSUM"))

    def phase_load(b):
        xs = []
        for k in range(2):
            t = px.tile([128, hw], MMDT, tag=f"xs{k}")
            for c in range(n_chunks):
                cs0 = slice(c * nc_cols, (c + 1) * nc_cols)
                nc.sync.dma_start(out=t[:, cs0],
                                  in_=dram["x"][b, 128 * k:128 * (k + 1), cs0])
            xs.append(t)
        return xs

    def means_alloc():
        return [pa.tile([128, 128], BF16, tag=f"xm{k}", bufs=4,
                        name=f"xm{k}") for k in range(2)]

    def means_piece(xs, xm, piece):
        # One of 4 pieces of the row/col sums of x: (dir, k). Identity-matmul
        # accumulation on PE (psum on the "att" tag so it never serializes
        # against the chunk psum rings), small reduce on DVE. xm[k] layout:
        # [128, 128] bf16, cols 0..63 = W-dir sums, 64..127 = H-dir sums.
        wt = max(w // 8, 1)
        wgroups = w // wt
        dirw, k = piece // 2, piece % 2
        if dirw == 0:
            psw = pp.tile([128, h * wt], F32, tag="att", name=f"ps_mw{k}")
            xv = xs[k].rearrange("p (h j t) -> p j h t", j=wgroups, t=wt)
            for j in range(wgroups):
                nc.tensor.matmul(psw, lhsT=ident128, rhs=xv[:, j],
                                 start=(j == 0), stop=(j == wgroups - 1))
            nc.vector.tensor_reduce(
                out=xm[k][:, 0:h].unsqueeze(-1),
                in_=psw.rearrange("p (h t) -> p h t", t=wt),
                axis=AX.X, op=ALU.add)
        else:
            psh = pp.tile([128, nc_cols], F32, tag="att", name=f"ps_mh{k}")
            for c in range(n_chunks):
                nc.tensor.matmul(
                    psh, lhsT=ident128,
                    rhs=xs[k][:, c * nc_cols:(c + 1) * nc_cols],
                    start=(c == 0), stop=(c == n_chunks - 1))
            nc.vector.tensor_reduce(
                out=xm[k][:, h:h + w].unsqueeze(-1),
                in_=psh.rearrange("p (s w) -> p w s", w=w),
                axis=AX.X, op=ALU.add)

    def phase_means(xs):
        xm = means_alloc()
        for piece in range(4):
            means_piece(xs, xm, piece)
        return xm

    def phase_attn(xm):
        """Both axial attentions. Returns (xr_f, xc_f): [2](128, nseq) bf16
        conv outputs + bias, pre-broadcast."""
        # q/k projections for BOTH directions at once:
        # qk psum [128, 512] = [q_t0 | q_t1 | k_t0 | k_t1], each 128 cols =
        # [dir-row 64 | dir-col 64]; padded head layout (16 kd + 16 zero).
        qk_ps = pp.tile([128, 512], F32, tag="att", name="qk_ps")
        for wi, wT in enumerate((wqTp, wkTp)):
            for t in range(2):
                sl = slice(256 * wi + 128 * t, 256 * wi + 128 * (t + 1))
                for k in range(2):
                    nc.tensor.matmul(qk_ps[:, sl],
                                     lhsT=wT[k][:, 128 * t:128 * (t + 1)],
                                     rhs=xm[k], start=(k == 0), stop=(k == 1))
        qk_sb = pa.tile([128, 512], BF16, tag="qk_sb")
        nc.vector.tensor_tensor(out=qk_sb, in0=qk_ps, in1=qkb, op=ALU.add)

        xatt = [None, None]
        for d in range(2):          # 0 = row (nseq=h), 1 = col (nseq=w)
            nseq = h if d == 0 else w
            do = 64 * d
            # scoresT [j, (h i)]. Heads h=j and h=j+4 share the partition
            # strip 32j; they go into one per-j psum tile (mixing different
            # tile_position rows inside one psum tile crashes the exec unit).
            eT = pa.tile([nseq, HEADS * nseq], BF16, tag=f"at_e{d}")
            eTv = eT.rearrange("p (t j i) -> p j t i", t=2, j=4)
            for j in range(4):
                st_ps = pp.tile([nseq, 2 * nseq], F32, tag="att",
                                name=f"st_ps{d}{j}")
                for t in range(2):
                    ksl = qk_sb[32 * j:32 * (j + 1),
                                256 + 128 * t + do:256 + 128 * t + do + 64]
                    qsl = qk_sb[32 * j:32 * (j + 1),
                                128 * t + do:128 * t + do + 64]
                    nc.tensor.matmul(st_ps[:, nseq * t:nseq * (t + 1)],
                                     lhsT=ksl, rhs=qsl, start=True, stop=True,
                                     tile_position=(32 * j, 0))
                nc.scalar.activation(
                    out=eTv[:, j], in_=st_ps.rearrange("p (t i) -> p t i", t=2),
                    func=AF.Exp, scale=KEY_DIM ** -0.5)
            # denominator row (scaled by W via onesW value) and reciprocal
            srow_ps = pp.tile([1, HEADS * nseq], F32, tag="att",
                              name=f"srow{d}")
            nc.tensor.matmul(srow_ps, lhsT=onesW[:nseq, :], rhs=eT,
                             start=True, stop=True)
            recip = pa.tile([1, HEADS * nseq], BF16, tag=f"at_rc{d}")
            nc.vector.reciprocal(out=recip, in_=srow_ps)
            rb_ps = pp.tile([nseq, HEADS * nseq], F32, tag="att",
                            name=f"rb{d}")
            nc.tensor.matmul(rb_ps, lhsT=ones1[:, :nseq], rhs=recip,
                             start=True, stop=True)
            eTn = pa.tile([nseq, HEADS * nseq], BF16, tag=f"at_en{d}")
            nc.vector.tensor_tensor(out=eTn, in0=eT, in1=rb_ps, op=ALU.mult)
            # vrT (nseq, 256) = xm_dir.T @ Wv.T
            vrT_ps = pp.tile([nseq, DH], F32, tag="att", name=f"vrT{d}")
            for k in range(2):
                nc.tensor.matmul(vrT_ps, lhsT=xm[k][:, do:do + nseq],
                                 rhs=wvTb[k], start=(k == 0), stop=(k == 1))
            vrT = pa.tile([nseq, DH], BF16, tag=f"at_vs{d}")
            nc.vector.tensor_copy(out=vrT, in_=vrT_ps)
            # attention out, transposed: xrT[i, 32h+d]
            xrT_ps = pp.tile([nseq, DH], F32, tag="att", name=f"xrT{d}")
            for hh in range(HEADS):
                nc.tensor.matmul(xrT_ps[:, 32 * hh:32 * (hh + 1)],
                                 lhsT=eTn[:, nseq * hh:nseq * (hh + 1)],
                                 rhs=vrT[:, 32 * hh:32 * (hh + 1)],
                                 start=True, stop=True)
            xrT_sb = pa.tile([nseq, DH], BF16, tag=f"at_xt{d}")
            nc.vector.tensor_copy(out=xrT_sb, in_=xrT_ps)
            # transpose back to (channel, i), relu(+bv) on eviction
            xr_relu = []
            for t in range(2):
                tr_ps = pp.tile([128, nseq], BF16, tag="att",
                                name=f"at_tr{d}_{t}")
                nc.tensor.transpose(tr_ps, xrT_sb[:, 128 * t:128 * (t + 1)],
                                    ident64[:nseq, :nseq])
                sb = pa.tile([128, nseq], BF16, tag=f"at_xrr{d}_{t}")
                nc.scalar.activation(out=sb, in_=tr_ps, func=AF.Relu,
                                     bias=bv_att[t])
                xr_relu.append(sb)
            # conv (dh -> dh) + bias
            wconvT = wrT if d == 0 else wcT
            bconv = brv if d == 0 else bcc
            xa = []
            for m in range(2):
                ps = pp.tile([128, nseq], F32, tag="att", name=f"at_cv{d}{m}")
                for k in range(2):
                    nc.tensor.matmul(ps,
                                     lhsT=wconvT[k][:, 128 * m:128 * (m + 1)],
                                     rhs=xr_relu[k],
                                     start=(k == 0), stop=(k == 1))
                sb = pa.tile([128, nseq], BF16, tag=f"at_xa{d}_{m}")
                nc.scalar.activation(out=sb, in_=ps, func=AF.Identity,
                                     bias=bconv[m])
                xa.append(sb)
            xatt[d] = xa
        return xatt[0], xatt[1]

    def phase_chunks(b, xs, xr_f, xc_f, c_lo=0, c_hi=None):
        for c in range(c_lo, c_hi if c_hi is not None else n_chunks):
            cs = slice(c * nc_cols, (c + 1) * nc_cols)
            hs = slice(c * chunk_h, (c + 1) * chunk_h)
            # q/k/v matmuls (q,k weights carry the depthwise gate scale)
            grp_ps = []
            for gi, (wT, mo) in enumerate(((wqT, 0), (wkT, 0),
                                           (wvT, 0), (wvT, 1))):
                ps = pp.tile([128, nc_cols], F32, tag=f"mm{gi}")
                for k in range(2):
                    nc.tensor.matmul(
                        ps, lhsT=wT[k][:, 128 * mo:128 * (mo + 1)],
                        rhs=xs[k][:, cs], start=(k == 0), stop=(k == 1))
                grp_ps.append(ps)
            # z evictions -> bf16 (q/k: relu+bias; v: relu+scale+bias)
            z = []
            for g in range(4):
                sb = pz.tile([128, nc_cols], MMDT, tag=f"z{g}")
                if g < 2:
                    nc.scalar.activation(out=sb, in_=grp_ps[g], func=AF.Relu,
                                         bias=zbias[g])
                else:
                    nc.scalar.activation(out=sb, in_=grp_ps[g], func=AF.Relu,
                                         scale=zsv[g - 2], bias=zbias[g])
                z.append(sb)
            # xx = relu(v + xr + xc) -> fp8 (i-major halves for DoubleRow)
            xx8 = pc.tile([128, 2, 512], FP8, tag="xx8")
            for m in range(2):
                rc = pc.tile([128, chunk_h, w], BF16, tag=f"rc{m}")
                nc.gpsimd.tensor_tensor(
                    out=rc,
                    in0=xr_f[m][:, hs].unsqueeze(-1).broadcast_to(
                        (128, chunk_h, w)),
                    in1=xc_f[m].unsqueeze(1).broadcast_to((128, chunk_h, w)),
                    op=ALU.add)
                t = pc.tile([128, nc_cols], BF16, tag=f"xx{m}")
                nc.vector.scalar_tensor_tensor(
                    out=t, in0=grp_ps[2 + m], scalar=0.0,
                    in1=rc.rearrange("p h w -> p (h w)"),
                    op0=ALU.add, op1=ALU.add)
                nc.vector.tensor_scalar(
                    out=xx8[:, m, :], in0=t,
                    scalar1=0.0, scalar2=0.0, op0=ALU.max, op1=ALU.add)
            # pointwise conv (512 -> 256) in bf16; bias folded into gate stt
            qkv_ps = []
            for m in range(2):
                ps = pp.tile([128, nc_cols], F32, tag=f"o{m}")
                for k in range(4):
                    nc.tensor.matmul(
                        ps, lhsT=wpwT[k][:, 128 * m:128 * (m + 1)],
                        rhs=z[k], start=(k == 0), stop=(k == 3))
                qkv_ps.append(ps)
            # final conv (256 -> 256) in fp8 DoubleRow; relu+bias on ACT;
            # gate stt on DVE
            for m in range(2):
                ps = pp.tile([128, nc_cols], F32, tag="p0", name=f"xp{m}")
                if USE_DR:
                    nc.tensor.matmul(ps, lhsT=wp8t[m], rhs=xx8,
                                     start=True, stop=True, perf_mode=DR)
                else:
                    for i in range(2):
                        nc.tensor.matmul(ps, lhsT=wp8t[m][:, i], rhs=xx8[:, i],
                                         start=(i == 0), stop=(i == 1))
                r = pc.tile([128, nc_cols], BF16, tag=f"r{m}")
                nc.scalar.activation(out=r, in_=ps, func=AF.Relu,
                                     scale=1.0 / 64.0, bias=bp3[m])
                r6 = pc.tile([128, nc_cols], BF16, tag=f"r6{m}")
                nc.vector.tensor_scalar(out=r6, in0=r, scalar1=6.0,
                                        scalar2=0.0, op0=ALU.min, op1=ALU.add)
                o = pout.tile([128, nc_cols], BF16, tag=f"ob{m}")
                nc.vector.scalar_tensor_tensor(
                    out=o, in0=qkv_ps[m], scalar=bpw6[m], in1=r6,
                    op0=ALU.add, op1=ALU.mult)
                ocs = slice(0, nc_cols) if tiny_out else cs
                nc.sync.dma_start(out=y_d[b, 128 * m:128 * (m + 1), ocs],
                                  in_=o)

    for _ in range(nrep):
        # Emission order = engine FIFO + tag-grant order. Chunks lead (they
        # only need x); means for the NEXT batch are spread as PE filler
        # between the previous batch's first-half chunks (their psums live on
        # the "att" tag so they never serialize against the chunk rings),
        # then attention runs before the second half.
        held = None
        half = n_chunks // 2
        for b in range(bpc):
            xs = phase_load(b)
            if held is None:
                xm = phase_means(xs)
            else:
                xm = means_alloc()
                for c in range(half):
                    phase_chunks(*held, c_lo=c, c_hi=c + 1)
                    means_piece(xs, xm, c)
            at = phase_attn(xm)
            if held is not None:
                phase_chunks(*held, c_lo=half)
            held = (b, xs, *at)
        phase_chunks(*held)


# ---------------------------------------------------------------------------
# host-side preparation
# ---------------------------------------------------------------------------

def _interp_pos_np(pe, n):
    s = pe.shape[-1]
    pos = np.clip((np.arange(n, dtype=np.float64) + 0.5) * (s / n) - 0.5,
                  0.0, s - 1.0).astype(np.float32)
    i0 = np.floor(pos).astype(np.int32)
    i1 = np.minimum(i0 + 1, s - 1)
    fw = (pos - i0).astype(np.float32)
    return pe[:, i0] * (1.0 - fw) + pe[:, i1] * fw


def prepare_consts(inputs, h=H, w=W, chunk_h=8):
    """Fold BN/scales and build the constant tensors the kernel expects."""
    import ml_dtypes
    f = lambda a: np.ascontiguousarray(np.asarray(a, dtype=np.float32))
    fb = lambda a: np.ascontiguousarray(
        np.asarray(a, dtype=np.float32).astype(ml_dtypes.bfloat16))
    Wq, sq, bq = f(inputs["Wq"]), f(inputs["sq"]), f(inputs["bq"])
    Wk, sk, bk = f(inputs["Wk"]), f(inputs["sk"]), f(inputs["bk"])
    Wv, sv, bv = f(inputs["Wv"]), f(inputs["sv"]), f(inputs["bv"])
    wdw, sdw, bdw = f(inputs["wdw"]), f(inputs["sdw"]), f(inputs["bdw"])
    Wpw, spw, bpw = f(inputs["Wpw"]), f(inputs["spw"]), f(inputs["bpw"])
    Wr, sr, br = f(inputs["Wr"]), f(inputs["sr"]), f(inputs["br"])
    Wc, sc, bc = f(inputs["Wc"]), f(inputs["sc"]), f(inputs["bc"])
    Wp, sp, bp = f(inputs["Wp"]), f(inputs["sp"]), f(inputs["bp"])

    Wq_f = sq[:, None] * Wq
    Wk_f = sk[:, None] * Wk
    Wv_f = sv[:, None] * Wv

    g = wdw * sdw
    bqkv = np.concatenate([bq, bk, bv])
    zbias = g * bqkv + bdw          # 512
    g_q, g_k, g_v = g[:NH_KD], g[NH_KD:2 * NH_KD], g[2 * NH_KD:]

    def tiles2(a):   # (256, cols) -> [(128, cols)] * 2
        return [a[:128], a[128:]]

    # q/k conv weights carry the depthwise gate scale
    wbig = np.concatenate(
        tiles2((g_q[None, :] * Wq_f.T)) + tiles2((g_k[None, :] * Wk_f.T))
        + tiles2(Wv_f.T)
        + [(((spw[:, None] * Wpw) / 6.0).T)[128 * k:128 * (k + 1)]
           for k in range(4)]
        + [np.eye(128, dtype=np.float32)], axis=1)
    consts = {"wbig": f(wbig)}
    # padded head layout for the attention q/k weights (1/mean fold included)
    assert h == w, "mean folds assume H == W"
    wqTp = np.zeros((DIM, 256), np.float32)
    wkTp = np.zeros((DIM, 256), np.float32)
    # qk bias layout matches the qk psum: [q_t0 | q_t1 | k_t0 | k_t1],
    # 128 cols each = [row-dir 64 | col-dir 64]; partitions 32h'+kd padded.
    qk_b = np.zeros((128, 512), np.float32)
    pe_rq = _interp_pos_np(f(inputs["pe_rq"]), h)
    pe_rk = _interp_pos_np(f(inputs["pe_rk"]), h)
    pe_cq = _interp_pos_np(f(inputs["pe_cq"]), w)
    pe_ck = _interp_pos_np(f(inputs["pe_ck"]), w)
    for hh in range(HEADS):
        sl_p = slice(32 * hh, 32 * hh + KEY_DIM)
        sl_c = slice(KEY_DIM * hh, KEY_DIM * (hh + 1))
        wqTp[:, sl_p] = (Wq_f[sl_c, :] / w).T
        wkTp[:, sl_p] = (Wk_f[sl_c, :] / w).T
        t, j = hh // 4, hh % 4
        prow = slice(32 * j, 32 * j + KEY_DIM)
        qk_b[prow, 128 * t:128 * t + 64] = bq[sl_c, None] + pe_rq[sl_c, :]
        qk_b[prow, 128 * t + 64:128 * t + 128] = bq[sl_c, None] + pe_cq[sl_c, :]
        qk_b[prow, 256 + 128 * t:256 + 128 * t + 64] = (
            bk[sl_c, None] + pe_rk[sl_c, :])
        qk_b[prow, 256 + 128 * t + 64:256 + 128 * t + 128] = (
            bk[sl_c, None] + pe_ck[sl_c, :])
    consts["qkbias"] = f(qk_b)
    id64pad = np.zeros((128, 64), np.float32)
    id64pad[:64] = np.eye(64, dtype=np.float32)
    wbigb = np.concatenate(
        tiles2((sr[:, None] * Wr).T) + tiles2((sc[:, None] * Wc).T)
        + tiles2(wqTp) + tiles2(wkTp) + tiles2(Wv_f.T)
        + [id64pad], axis=1)
    consts["wbigb"] = fb(wbigb)
    # fp8 DoubleRow final-conv weights: wp8[p, i*256 + m] = 64*Wp_f[m, p+128i]
    Wp_f = sp[:, None] * Wp
    wp8 = np.zeros((128, 512), np.float32)
    for mo in range(2):
        for i in range(2):
            wp8[:, 256 * mo + 128 * i:256 * mo + 128 * (i + 1)] = (
                64.0 * Wp_f[128 * mo:128 * (mo + 1),
                            128 * i:128 * (i + 1)].T)
    consts["wp8"] = np.ascontiguousarray(wp8.astype(ml_dtypes.float8_e4m3))

    params = np.zeros((128, 20), np.float32)
    params[:, 0:2] = g_v.reshape(2, 128).T
    params[:, 2:6] = zbias.reshape(4, 128).T
    params[:, 6:8] = bv.reshape(2, 128).T
    params[:, 8:10] = (br + bv).reshape(2, 128).T
    params[:, 10:12] = bc.reshape(2, 128).T
    params[:, 12:14] = (bp + 3.0).reshape(2, 128).T
    params[:, 14:16] = (bpw / 6.0).reshape(2, 128).T
    consts["params"] = f(params)
    consts["onesW"] = np.full((max(h, w), 1), float(w),
                              ml_dtypes.bfloat16)
    consts["ones1"] = np.ones((1, 64), ml_dtypes.bfloat16)
    return consts


_NC_CACHE = {}


def _get_nc():
    if "nc" not in _NC_CACHE:
        _NC_CACHE["nc"] = build_nc()
    return _NC_CACHE["nc"]


def kernel(**inputs) -> np.ndarray:
    x = np.ascontiguousarray(np.asarray(inputs["x"], dtype=np.float32))
    consts = prepare_consts(inputs)
    nc = _get_nc()
    in_maps = []
    for c in range(N_CORES):
        m = dict(consts)
        m["x"] = np.ascontiguousarray(
            x[c * BPC:(c + 1) * BPC].reshape(BPC, DIM, H * W))
        in_maps.append(m)
    res = bass_utils.run_bass_kernel_spmd(nc, in_maps,
                                          core_ids=list(range(N_CORES)))
    y = np.concatenate([np.asarray(r["y"], dtype=np.float32)
                        for r in res.results], axis=0)
    return y.reshape(B, DIM, H, W)


# revision 15
# speedup vs baseline: 1.0459x; 1.0459x over previous
# Trainium2 Bass kernel for the ASE (axial squeeze attention) block.
#
# Sharding: pure data parallel over batch B=16 across 8 NeuronCores
# (2 batches per core); all params replicated.
#
# Math restructuring (host-side folds):
#  - BN scales fold into conv weights; biases applied during PSUM evictions.
#  - depthwise gate scale g folds into the q/k conv weights (diagonal
#    commutes); v keeps g on its ACT eviction since the raw v psum also
#    feeds the xx path.
#  - 1x1 convs commute with spatial means, so row/col attention only needs
#    the row/col sums of x (256x64 each), never full q/k maps.
#  - positional embeddings interpolated on host and folded into qk biases.
#  - softmax: exp on ACT; denominator via ones-matmul; normalization by
#    broadcasting the reciprocal row with a K=1 matmul.
#  - h_sigmoid(x)*gate = min(relu(x+3), 6) * (gate/6): the /6 folds into the
#    pointwise conv weights; the final conv runs in fp8e4 DoubleRow perf
#    mode (weights x64 to stay in fp8 normal range; /64 on the ACT evict).
#  - y stored bf16, upcast on host.
import numpy as np

import concourse.bass as bass
import concourse.mybir as mybir
import concourse.tile as tile
from concourse import bacc, bass_utils

F32 = mybir.dt.float32
F32R = mybir.dt.float32r
BF16 = mybir.dt.bfloat16
FP8 = mybir.dt.float8e4
AF = mybir.ActivationFunctionType
ALU = mybir.AluOpType
AX = mybir.AxisListType
DR = mybir.MatmulPerfMode.DoubleRow
USE_DR = True
SKIP_ATTN = False

B, DIM, H, W = 16, 256, 64, 64
KEY_DIM, HEADS = 16, 8
NH_KD = KEY_DIM * HEADS   # 128
DH = 2 * KEY_DIM * HEADS  # 256
POS = 16
N_CORES = 8
BPC = B // N_CORES        # batches per core

MMDT = F32R  # dtype of x / qkv weights feeding the PE

WBIG_COLS = 128 * 2 + 128 * 2 + 256 * 2 + 256 * 4   # q,k,v,pw
WBIGB_COLS = 512 + 512 + 512 + 512 + 512 + 64   # wr,wc,wq,wk,wv,id64


def build_nc(bpc=BPC, h=H, w=W, chunk_h=8, num_devices=N_CORES, use_f32r=True,
             nrep=1, tiny_out=False):
    """Build the per-core Bass module."""
    global MMDT
    MMDT = F32R if use_f32r else F32
    hw = h * w
    n_chunks = h // chunk_h
    nc_cols = chunk_h * w      # spatial columns per chunk

    nc = bacc.Bacc("TRN2", target_bir_lowering=False, debug=False,
                   num_devices=num_devices)

    dram = {}

    def din(name, shape, dt=None):
        dram[name] = nc.dram_tensor(name, shape, dt or F32,
                                    kind="ExternalInput").ap()
        return dram[name]

    din("x", (bpc, DIM, hw), MMDT)
    din("ident", (128, 128), MMDT)
    din("wbig", (128, WBIG_COLS), MMDT)
    din("wbigb", (128, WBIGB_COLS), BF16)
    din("wp8", (128, 512), FP8)
    din("qkbias", (128, 512))
    din("params", (128, 20))
    din("onesW", (max(h, w), 1), BF16)
    din("ones1", (1, 64), BF16)
    y_cols = nc_cols if tiny_out else hw
    y_d = nc.dram_tensor("y", (bpc, DIM, y_cols), BF16,
                         kind="ExternalOutput").ap()

    with tile.TileContext(nc) as tc:
        _emit(nc, tc, dram, y_d, bpc, h, w, hw, chunk_h, n_chunks, nc_cols,
              nrep, tiny_out)
    nc.compile()
    return nc


def _emit(nc, tc, dram, y_d, bpc, h, w, hw, chunk_h, n_chunks, nc_cols,
          nrep=1, tiny_out=False):
    from contextlib import ExitStack
    with ExitStack() as _ctx:
        _emit_body(_ctx, nc, tc, dram, y_d, bpc, h, w, hw, chunk_h, n_chunks,
                   nc_cols, nrep, tiny_out)


def _emit_body(ctx, nc, tc, dram, y_d, bpc, h, w, hw, chunk_h, n_chunks,
               nc_cols, nrep=1, tiny_out=False):
    ctx.enter_context(nc.allow_low_precision(
        reason="bf16/fp8 matmul operand rounding"))
    # ---- persistent weights / params (loaded once) ----
    wp = ctx.enter_context(tc.tile_pool(name="weights", bufs=1))

    ident128 = wp.tile([128, 128], MMDT, tag="ident", name="ident128")
    nc.scalar.dma_start(out=ident128, in_=dram["ident"])
    wbig = wp.tile([128, WBIG_COLS], MMDT, tag="wbig")
    nc.scalar.dma_start(out=wbig, in_=dram["wbig"])
    wbigb = wp.tile([128, WBIGB_COLS], BF16, tag="wbigb")
    nc.scalar.dma_start(out=wbigb, in_=dram["wbigb"])
    # DoubleRow stationary tiles must be native [p, 2, m] 3-D tiles
    wp8t = []
    for mo in range(2):
        t8 = wp.tile([128, 2, 128], FP8, tag=f"wp8_{mo}", name=f"wp8_{mo}")
        nc.scalar.dma_start(out=t8.rearrange("p i f -> p (i f)"),
                          in_=dram["wp8"][:, 256 * mo:256 * (mo + 1)])
        wp8t.append(t8)
    params = wp.tile([128, 20], F32, tag="params")
    nc.scalar.dma_start(out=params, in_=dram["params"])
    qkb = wp.tile([128, 512], F32, tag="qkb")
    nc.scalar.dma_start(out=qkb, in_=dram["qkbias"])

    def _slices(tile_, widths):
        out, off = [], 0
        for wd in widths:
            out.append(tile_[:, off:off + wd])
            off += wd
        return out

    (wqT0, wqT1, wkT0, wkT1, wvT0, wvT1, wpw0, wpw1, wpw2, wpw3) = _slices(
        wbig, [NH_KD, NH_KD, NH_KD, NH_KD, DH, DH,
               DIM, DIM, DIM, DIM])
    wqT, wkT, wvT = [wqT0, wqT1], [wkT0, wkT1], [wvT0, wvT1]
    wpwT = [wpw0, wpw1, wpw2, wpw3]
    (wrT0, wrT1, wcT0, wcT1, wqTp0, wqTp1, wkTp0, wkTp1, wvTb0, wvTb1,
     ident64b) = _slices(
        wbigb, [DH, DH, DH, DH, 256, 256, 256, 256, DH, DH, 64])
    wrT, wcT = [wrT0, wrT1], [wcT0, wcT1]
    wqTp, wkTp, wvTb = [wqTp0, wqTp1], [wkTp0, wkTp1], [wvTb0, wvTb1]
    ident64 = ident64b[:64, :]
    onesW = wp.tile([max(h, w), 1], BF16, tag="onesW")   # value = W (mean fold)
    nc.scalar.dma_start(out=onesW, in_=dram["onesW"])
    ones1 = wp.tile([1, 64], BF16, tag="ones1")
    nc.scalar.dma_start(out=ones1, in_=dram["ones1"])

    # param columns
    zsv = [params[:, g:g + 1] for g in range(2)]          # zscale for v grps
    zbias = [params[:, 2 + g:3 + g] for g in range(4)]    # q, k, v0, v1
    bv_att = [params[:, 6 + m:7 + m] for m in range(2)]
    brv = [params[:, 8 + m:9 + m] for m in range(2)]
    bcc = [params[:, 10 + m:11 + m] for m in range(2)]
    bp3 = [params[:, 12 + m:13 + m] for m in range(2)]
    bpw6 = [params[:, 14 + m:15 + m] for m in range(2)]

    # ---- pools ----
    px = ctx.enter_context(tc.tile_pool(name="x", bufs=2))
    pa = ctx.enter_context(tc.tile_pool(name="attn", bufs=1))
    pz = ctx.enter_context(tc.tile_pool(name="z", bufs=5))
    pc = ctx.enter_context(tc.tile_pool(name="chunk", bufs=3))
    pout = ctx.enter_context(tc.tile_pool(name="outb", bufs=3))
    pp = ctx.enter_context(tc.tile_pool(name="psum", bufs=1, space="PSUM"))

    def phase_load(b):
        xs = []
        for k in range(2):
            t = px.tile([128, hw], MMDT, tag=f"xs{k}")
            for c in range(n_chunks):
                cs0 = slice(c * nc_cols, (c + 1) * nc_cols)
                nc.sync.dma_start(out=t[:, cs0],
                                  in_=dram["x"][b, 128 * k:128 * (k + 1), cs0])
            xs.append(t)
        return xs

    def means_alloc():
        return [pa.tile([128, 128], BF16, tag=f"xm{k}", bufs=4,
                        name=f"xm{k}") for k in range(2)]

    def means_piece(xs, xm, piece):
        # One of 4 pieces of the row/col sums of x: (dir, k). Identity-matmul
        # accumulation on PE (psum on the "att" tag so it never serializes
        # against the chunk psum rings), small reduce on DVE. xm[k] layout:
        # [128, 128] bf16, cols 0..63 = W-dir sums, 64..127 = H-dir sums.
        wt = max(w // 8, 1)
        wgroups = w // wt
        dirw, k = piece // 2, piece % 2
        ptag = "att" if piece % 2 == 0 else "p0"
        if dirw == 0:
            psw = pp.tile([128, h * wt], F32, tag=ptag, name=f"ps_mw{k}")
            xv = xs[k].rearrange("p (h j t) -> p j h t", j=wgroups, t=wt)
            for j in range(wgroups):
                nc.tensor.matmul(psw, lhsT=ident128, rhs=xv[:, j],
                                 start=(j == 0), stop=(j == wgroups - 1))
            with tc.high_priority(offset=64):
                nc.vector.tensor_reduce(
                    out=xm[k][:, 0:h].unsqueeze(-1),
                    in_=psw.rearrange("p (h t) -> p h t", t=wt),
                    axis=AX.X, op=ALU.add)
        else:
            psh = pp.tile([128, nc_cols], F32, tag=ptag, name=f"ps_mh{k}")
            for c in range(n_chunks):
                nc.tensor.matmul(
                    psh, lhsT=ident128,
                    rhs=xs[k][:, c * nc_cols:(c + 1) * nc_cols],
                    start=(c == 0), stop=(c == n_chunks - 1))
            with tc.high_priority(offset=64):
                nc.vector.tensor_reduce(
                    out=xm[k][:, h:h + w].unsqueeze(-1),
                    in_=psh.rearrange("p (s w) -> p w s", w=w),
                    axis=AX.X, op=ALU.add)

    def phase_means(xs):
        xm = means_alloc()
        for piece in range(4):
            means_piece(xs, xm, piece)
        return xm

    def phase_attn(xm):
        """Both axial attentions. Returns (xr_f, xc_f): [2](128, nseq) bf16
        conv outputs + bias, pre-broadcast."""
        # q/k projections for BOTH directions at once:
        # qk psum [128, 512] = [q_t0 | q_t1 | k_t0 | k_t1], each 128 cols =
        # [dir-row 64 | dir-col 64]; padded head layout (16 kd + 16 zero).
        qk_ps = pp.tile([128, 512], F32, tag="att", name="qk_ps")
        for wi, wT in enumerate((wqTp, wkTp)):
            for t in range(2):
                sl = slice(256 * wi + 128 * t, 256 * wi + 128 * (t + 1))
                for k in range(2):
                    nc.tensor.matmul(qk_ps[:, sl],
                                     lhsT=wT[k][:, 128 * t:128 * (t + 1)],
                                     rhs=xm[k], start=(k == 0), stop=(k == 1))
        qk_sb = pa.tile([128, 512], BF16, tag="qk_sb")
        nc.vector.tensor_tensor(out=qk_sb, in0=qk_ps, in1=qkb, op=ALU.add)

        xatt = [None, None]
        for d in range(2):          # 0 = row (nseq=h), 1 = col (nseq=w)
            nseq = h if d == 0 else w
            do = 64 * d
            # scoresT [j, (h i)]. Heads h=j and h=j+4 share the partition
            # strip 32j; they go into one per-j psum tile (mixing different
            # tile_position rows inside one psum tile crashes the exec unit).
            eT = pa.tile([nseq, HEADS * nseq], BF16, tag=f"at_e{d}")
            eTv = eT.rearrange("p (t j i) -> p j t i", t=2, j=4)
            for j in range(4):
                st_ps = pp.tile([nseq, 2 * nseq], F32, tag="att",
                                name=f"st_ps{d}{j}")
                for t in range(2):
                    ksl = qk_sb[32 * j:32 * (j + 1),
                                256 + 128 * t + do:256 + 128 * t + do + 64]
                    qsl = qk_sb[32 * j:32 * (j + 1),
                                128 * t + do:128 * t + do + 64]
                    nc.tensor.matmul(st_ps[:, nseq * t:nseq * (t + 1)],
                                     lhsT=ksl, rhs=qsl, start=True, stop=True,
                                     tile_position=(32 * j, 0))
                nc.scalar.activation(
                    out=eTv[:, j], in_=st_ps.rearrange("p (t i) -> p t i", t=2),
                    func=AF.Exp, scale=KEY_DIM ** -0.5)
            # denominator row (scaled by W via onesW value) and reciprocal
            srow_ps = pp.tile([1, HEADS * nseq], F32, tag="att",
                              name=f"srow{d}")
            nc.tensor.matmul(srow_ps, lhsT=onesW[:nseq, :], rhs=eT,
                             start=True, stop=True)
            recip = pa.tile([1, HEADS * nseq], BF16, tag=f"at_rc{d}")
            nc.vector.reciprocal(out=recip, in_=srow_ps)
            rb_ps = pp.tile([nseq, HEADS * nseq], F32, tag="att",
                            name=f"rb{d}")
            nc.tensor.matmul(rb_ps, lhsT=ones1[:, :nseq], rhs=recip,
                             start=True, stop=True)
            eTn = pa.tile([nseq, HEADS * nseq], BF16, tag=f"at_en{d}")
            nc.vector.tensor_tensor(out=eTn, in0=eT, in1=rb_ps, op=ALU.mult)
            # vrT (nseq, 256) = xm_dir.T @ Wv.T
            vrT_ps = pp.tile([nseq, DH], F32, tag="att", name=f"vrT{d}")
            for k in range(2):
                nc.tensor.matmul(vrT_ps, lhsT=xm[k][:, do:do + nseq],
                                 rhs=wvTb[k], start=(k == 0), stop=(k == 1))
            vrT = pa.tile([nseq, DH], BF16, tag=f"at_vs{d}")
            nc.vector.tensor_copy(out=vrT, in_=vrT_ps)
            # attention out, transposed: xrT[i, 32h+d]
            xrT_ps = pp.tile([nseq, DH], F32, tag="att", name=f"xrT{d}")
            for hh in range(HEADS):
                nc.tensor.matmul(xrT_ps[:, 32 * hh:32 * (hh + 1)],
                                 lhsT=eTn[:, nseq * hh:nseq * (hh + 1)],
                                 rhs=vrT[:, 32 * hh:32 * (hh + 1)],
                                 start=True, stop=True)
            xrT_sb = pa.tile([nseq, DH], BF16, tag=f"at_xt{d}")
            nc.vector.tensor_copy(out=xrT_sb, in_=xrT_ps)
            # transpose back to (channel, i), relu(+bv) on eviction
            xr_relu = []
            for t in range(2):
                tr_ps = pp.tile([128, nseq], BF16, tag="att",
                                name=f"at_tr{d}_{t}")
                nc.tensor.transpose(tr_ps, xrT_sb[:, 128 * t:128 * (t + 1)],
                                    ident64[:nseq, :nseq])
                sb = pa.tile([128, nseq], BF16, tag=f"at_xrr{d}_{t}")
                nc.scalar.activation(out=sb, in_=tr_ps, func=AF.Relu,
                                     bias=bv_att[t])
                xr_relu.append(sb)
            # conv (dh -> dh) + bias
            wconvT = wrT if d == 0 else wcT
            bconv = brv if d == 0 else bcc
            xa = []
            for m in range(2):
                ps = pp.tile([128, nseq], F32, tag="att", name=f"at_cv{d}{m}")
                for k in range(2):
                    nc.tensor.matmul(ps,
                                     lhsT=wconvT[k][:, 128 * m:128 * (m + 1)],
                                     rhs=xr_relu[k],
                                     start=(k == 0), stop=(k == 1))
                sb = pa.tile([128, nseq], BF16, tag=f"at_xa{d}_{m}")
                nc.scalar.activation(out=sb, in_=ps, func=AF.Identity,
                                     bias=bconv[m])
                xa.append(sb)
            xatt[d] = xa
        return xatt[0], xatt[1]

    def phase_chunks(b, xs, xr_f, xc_f, c_lo=0, c_hi=None):
        for c in range(c_lo, c_hi if c_hi is not None else n_chunks):
            cs = slice(c * nc_cols, (c + 1) * nc_cols)
            hs = slice(c * chunk_h, (c + 1) * chunk_h)
            # q/k/v matmuls (q,k weights carry the depthwise gate scale)
            grp_ps = []
            for gi, (wT, mo) in enumerate(((wqT, 0), (wkT, 0),
                                           (wvT, 0), (wvT, 1))):
                ps = pp.tile([128, nc_cols], F32, tag=f"mm{gi}")
                for k in range(2):
                    nc.tensor.matmul(
                        ps, lhsT=wT[k][:, 128 * mo:128 * (mo + 1)],
                        rhs=xs[k][:, cs], start=(k == 0), stop=(k == 1))
                grp_ps.append(ps)
            # z evictions -> bf16 (q/k: relu+bias; v: relu+scale+bias)
            z = []
            for g in range(4):
                sb = pz.tile([128, nc_cols], MMDT, tag=f"z{g}")
                if g < 2:
                    nc.scalar.activation(out=sb, in_=grp_ps[g], func=AF.Relu,
                                         bias=zbias[g])
                else:
                    nc.scalar.activation(out=sb, in_=grp_ps[g], func=AF.Relu,
                                         scale=zsv[g - 2], bias=zbias[g])
                z.append(sb)
            # xx = relu(v + xr + xc) -> fp8 (i-major halves for DoubleRow)
            xx8 = pc.tile([128, 2, 512], FP8, tag="xx8")
            for m in range(2):
                rc = pc.tile([128, chunk_h, w], BF16, tag=f"rc{m}")
                nc.gpsimd.tensor_tensor(
                    out=rc,
                    in0=xr_f[m][:, hs].unsqueeze(-1).broadcast_to(
                        (128, chunk_h, w)),
                    in1=xc_f[m].unsqueeze(1).broadcast_to((128, chunk_h, w)),
                    op=ALU.add)
                t = pc.tile([128, nc_cols], BF16, tag=f"xx{m}")
                with tc.high_priority(offset=64):
                    nc.vector.scalar_tensor_tensor(
                        out=t, in0=grp_ps[2 + m], scalar=0.0,
                        in1=rc.rearrange("p h w -> p (h w)"),
                        op0=ALU.add, op1=ALU.add)
                    nc.vector.tensor_scalar(
                        out=xx8[:, m, :], in0=t,
                        scalar1=0.0, scalar2=0.0, op0=ALU.max, op1=ALU.add)
            # pointwise conv (512 -> 256) in bf16; bias folded into gate stt
            qkv_ps = []
            for m in range(2):
                ps = pp.tile([128, nc_cols], F32, tag=f"o{m}")
                for k in range(4):
                    nc.tensor.matmul(
                        ps, lhsT=wpwT[k][:, 128 * m:128 * (m + 1)],
                        rhs=z[k], start=(k == 0), stop=(k == 3))
                qkv_ps.append(ps)
            # final conv (256 -> 256) in fp8 DoubleRow; relu+bias on ACT;
            # gate stt on DVE
            for m in range(2):
                ps = pp.tile([128, nc_cols], F32, tag="p0", name=f"xp{m}")
                if USE_DR:
                    nc.tensor.matmul(ps, lhsT=wp8t[m], rhs=xx8,
                                     start=True, stop=True, perf_mode=DR)
                else:
                    for i in range(2):
                        nc.tensor.matmul(ps, lhsT=wp8t[m][:, i], rhs=xx8[:, i],
                                         start=(i == 0), stop=(i == 1))
                r = pc.tile([128, nc_cols], BF16, tag=f"r{m}")
                nc.scalar.activation(out=r, in_=ps, func=AF.Relu,
                                     scale=1.0 / 64.0, bias=bp3[m])
                r6 = pc.tile([128, nc_cols], BF16, tag=f"r6{m}")
                nc.vector.tensor_scalar(out=r6, in0=r, scalar1=6.0,
                                        scalar2=0.0, op0=ALU.min, op1=ALU.add)
                o = pout.tile([128, nc_cols], BF16, tag=f"ob{m}")
                nc.vector.scalar_tensor_tensor(
                    out=o, in0=qkv_ps[m], scalar=bpw6[m], in1=r6,
                    op0=ALU.add, op1=ALU.mult)
                ocs = slice(0, nc_cols) if tiny_out else cs
                nc.sync.dma_start(out=y_d[b, 128 * m:128 * (m + 1), ocs],
                                  in_=o)

    for _ in range(nrep):
        # Emission order = engine FIFO + tag-grant order. Chunks lead (they
        # only need x); means for the NEXT batch are spread as PE filler
        # between the previous batch's first-half chunks (their psums live on
        # the "att" tag so they never serialize against the chunk rings),
        # then attention runs before the second half.
        held = None
        half = n_chunks // 2
        for b in range(bpc):
            xs = phase_load(b)
            if held is None:
                xm = phase_means(xs)
            else:
                xm = means_alloc()
                for c in range(half):
                    phase_chunks(*held, c_lo=c, c_hi=c + 1)
                    means_piece(xs, xm, c)
            at = phase_attn(xm)
            if held is not None:
                phase_chunks(*held, c_lo=half)
            held = (b, xs, *at)
        phase_chunks(*held)


# ---------------------------------------------------------------------------
# host-side preparation
# ---------------------------------------------------------------------------

def _interp_pos_np(pe, n):
    s = pe.shape[-1]
    pos = np.clip((np.arange(n, dtype=np.float64) + 0.5) * (s / n) - 0.5,
                  0.0, s - 1.0).astype(np.float32)
    i0 = np.floor(pos).astype(np.int32)
    i1 = np.minimum(i0 + 1, s - 1)
    fw = (pos - i0).astype(np.float32)
    return pe[:, i0] * (1.0 - fw) + pe[:, i1] * fw


def prepare_consts(inputs, h=H, w=W, chunk_h=8):
    """Fold BN/scales and build the constant tensors the kernel expects."""
    import ml_dtypes
    f = lambda a: np.ascontiguousarray(np.asarray(a, dtype=np.float32))
    fb = lambda a: np.ascontiguousarray(
        np.asarray(a, dtype=np.float32).astype(ml_dtypes.bfloat16))
    Wq, sq, bq = f(inputs["Wq"]), f(inputs["sq"]), f(inputs["bq"])
    Wk, sk, bk = f(inputs["Wk"]), f(inputs["sk"]), f(inputs["bk"])
    Wv, sv, bv = f(inputs["Wv"]), f(inputs["sv"]), f(inputs["bv"])
    wdw, sdw, bdw = f(inputs["wdw"]), f(inputs["sdw"]), f(inputs["bdw"])
    Wpw, spw, bpw = f(inputs["Wpw"]), f(inputs["spw"]), f(inputs["bpw"])
    Wr, sr, br = f(inputs["Wr"]), f(inputs["sr"]), f(inputs["br"])
    Wc, sc, bc = f(inputs["Wc"]), f(inputs["sc"]), f(inputs["bc"])
    Wp, sp, bp = f(inputs["Wp"]), f(inputs["sp"]), f(inputs["bp"])

    Wq_f = sq[:, None] * Wq
    Wk_f = sk[:, None] * Wk
    Wv_f = sv[:, None] * Wv

    g = wdw * sdw
    bqkv = np.concatenate([bq, bk, bv])
    zbias = g * bqkv + bdw          # 512
    g_q, g_k, g_v = g[:NH_KD], g[NH_KD:2 * NH_KD], g[2 * NH_KD:]

    def tiles2(a):   # (256, cols) -> [(128, cols)] * 2
        return [a[:128], a[128:]]

    # q/k conv weights carry the depthwise gate scale
    wbig = np.concatenate(
        tiles2((g_q[None, :] * Wq_f.T)) + tiles2((g_k[None, :] * Wk_f.T))
        + tiles2(Wv_f.T)
        + [(((spw[:, None] * Wpw) / 6.0).T)[128 * k:128 * (k + 1)]
           for k in range(4)], axis=1)
    consts = {"wbig": f(wbig), "ident": np.eye(128, dtype=np.float32)}
    # padded head layout for the attention q/k weights (1/mean fold included)
    assert h == w, "mean folds assume H == W"
    wqTp = np.zeros((DIM, 256), np.float32)
    wkTp = np.zeros((DIM, 256), np.float32)
    # qk bias layout matches the qk psum: [q_t0 | q_t1 | k_t0 | k_t1],
    # 128 cols each = [row-dir 64 | col-dir 64]; partitions 32h'+kd padded.
    qk_b = np.zeros((128, 512), np.float32)
    pe_rq = _interp_pos_np(f(inputs["pe_rq"]), h)
    pe_rk = _interp_pos_np(f(inputs["pe_rk"]), h)
    pe_cq = _interp_pos_np(f(inputs["pe_cq"]), w)
    pe_ck = _interp_pos_np(f(inputs["pe_ck"]), w)
    for hh in range(HEADS):
        sl_p = slice(32 * hh, 32 * hh + KEY_DIM)
        sl_c = slice(KEY_DIM * hh, KEY_DIM * (hh + 1))
        wqTp[:, sl_p] = (Wq_f[sl_c, :] / w).T
        wkTp[:, sl_p] = (Wk_f[sl_c, :] / w).T
        t, j = hh // 4, hh % 4
        prow = slice(32 * j, 32 * j + KEY_DIM)
        qk_b[prow, 128 * t:128 * t + 64] = bq[sl_c, None] + pe_rq[sl_c, :]
        qk_b[prow, 128 * t + 64:128 * t + 128] = bq[sl_c, None] + pe_cq[sl_c, :]
        qk_b[prow, 256 + 128 * t:256 + 128 * t + 64] = (
            bk[sl_c, None] + pe_rk[sl_c, :])
        qk_b[prow, 256 + 128 * t + 64:256 + 128 * t + 128] = (
            bk[sl_c, None] + pe_ck[sl_c, :])
    consts["qkbias"] = f(qk_b)
    id64pad = np.zeros((128, 64), np.float32)
    id64pad[:64] = np.eye(64, dtype=np.float32)
    wbigb = np.concatenate(
        tiles2((sr[:, None] * Wr).T) + tiles2((sc[:, None] * Wc).T)
        + tiles2(wqTp) + tiles2(wkTp) + tiles2(Wv_f.T)
        + [id64pad], axis=1)
    consts["wbigb"] = fb(wbigb)
    # fp8 DoubleRow final-conv weights: wp8[p, i*256 + m] = 64*Wp_f[m, p+128i]
    Wp_f = sp[:, None] * Wp
    wp8 = np.zeros((128, 512), np.float32)
    for mo in range(2):
        for i in range(2):
            wp8[:, 256 * mo + 128 * i:256 * mo + 128 * (i + 1)] = (
                64.0 * Wp_f[128 * mo:128 * (mo + 1),
                            128 * i:128 * (i + 1)].T)
    consts["wp8"] = np.ascontiguousarray(wp8.astype(ml_dtypes.float8_e4m3))

    params = np.zeros((128, 20), np.float32)
    params[:, 0:2] = g_v.reshape(2, 128).T
    params[:, 2:6] = zbias.reshape(4, 128).T
    params[:, 6:8] = bv.reshape(2, 128).T
    params[:, 8:10] = (br + bv).reshape(2, 128).T
    params[:, 10:12] = bc.reshape(2, 128).T
    params[:, 12:14] = (bp + 3.0).reshape(2, 128).T
    params[:, 14:16] = (bpw / 6.0).reshape(2, 128).T
    consts["params"] = f(params)
    consts["onesW"] = np.full((max(h, w), 1), float(w),
                              ml_dtypes.bfloat16)
    consts["ones1"] = np.ones((1, 64), ml_dtypes.bfloat16)
    return consts


_NC_CACHE = {}


def _get_nc():
    if "nc" not in _NC_CACHE:
        _NC_CACHE["nc"] = build_nc()
    return _NC_CACHE["nc"]


def kernel(**inputs) -> np.ndarray:
    x = np.ascontiguousarray(np.asarray(inputs["x"], dtype=np.float32))
    consts = prepare_consts(inputs)
    nc = _get_nc()
    in_maps = []
    for c in range(N_CORES):
        m = dict(consts)
        m["x"] = np.ascontiguousarray(
            x[c * BPC:(c + 1) * BPC].reshape(BPC, DIM, H * W))
        in_maps.append(m)
    res = bass_utils.run_bass_kernel_spmd(nc, in_maps,
                                          core_ids=list(range(N_CORES)))
    y = np.concatenate([np.asarray(r["y"], dtype=np.float32)
                        for r in res.results], axis=0)
    return y.reshape(B, DIM, H, W)


# revision 16
# speedup vs baseline: 1.0461x; 1.0002x over previous
# Trainium2 Bass kernel for the ASE (axial squeeze attention) block.
#
# Sharding: pure data parallel over batch B=16 across 8 NeuronCores
# (2 batches per core); all params replicated.
#
# Math restructuring (host-side folds):
#  - BN scales fold into conv weights; biases applied during PSUM evictions.
#  - depthwise gate scale g folds into the q/k conv weights (diagonal
#    commutes); v keeps g on its ACT eviction since the raw v psum also
#    feeds the xx path.
#  - 1x1 convs commute with spatial means, so row/col attention only needs
#    the row/col sums of x (256x64 each), never full q/k maps.
#  - positional embeddings interpolated on host and folded into qk biases.
#  - softmax: exp on ACT; denominator via ones-matmul; normalization by
#    broadcasting the reciprocal row with a K=1 matmul.
#  - h_sigmoid(x)*gate = min(relu(x+3), 6) * (gate/6): the /6 folds into the
#    pointwise conv weights; the final conv runs in fp8e4 DoubleRow perf
#    mode (weights x64 to stay in fp8 normal range; /64 on the ACT evict).
#  - y stored bf16, upcast on host.
import numpy as np

import concourse.bass as bass
import concourse.mybir as mybir
import concourse.tile as tile
from concourse import bacc, bass_utils

F32 = mybir.dt.float32
F32R = mybir.dt.float32r
BF16 = mybir.dt.bfloat16
FP8 = mybir.dt.float8e4
AF = mybir.ActivationFunctionType
ALU = mybir.AluOpType
AX = mybir.AxisListType
DR = mybir.MatmulPerfMode.DoubleRow
USE_DR = True
SKIP_ATTN = False

B, DIM, H, W = 16, 256, 64, 64
KEY_DIM, HEADS = 16, 8
NH_KD = KEY_DIM * HEADS   # 128
DH = 2 * KEY_DIM * HEADS  # 256
POS = 16
N_CORES = 8
BPC = B // N_CORES        # batches per core

MMDT = F32R  # dtype of x / qkv weights feeding the PE

WBIG_COLS = 128 * 2 + 128 * 2 + 256 * 2 + 256 * 4   # q,k,v,pw
WBIGB_COLS = 512 + 512 + 512 + 512 + 512 + 64   # wr,wc,wq,wk,wv,id64


def build_nc(bpc=BPC, h=H, w=W, chunk_h=8, num_devices=N_CORES, use_f32r=True,
             nrep=1, tiny_out=False):
    """Build the per-core Bass module."""
    global MMDT
    MMDT = F32R if use_f32r else F32
    hw = h * w
    n_chunks = h // chunk_h
    nc_cols = chunk_h * w      # spatial columns per chunk

    nc = bacc.Bacc("TRN2", target_bir_lowering=False, debug=False,
                   num_devices=num_devices)

    dram = {}

    def din(name, shape, dt=None):
        dram[name] = nc.dram_tensor(name, shape, dt or F32,
                                    kind="ExternalInput").ap()
        return dram[name]

    din("x", (bpc, DIM, hw), MMDT)
    din("ident", (128, 128), MMDT)
    din("wbig", (128, WBIG_COLS), MMDT)
    din("wbigb", (128, WBIGB_COLS), BF16)
    din("wp8", (128, 512), FP8)
    din("qkbias", (128, 512))
    din("params", (128, 20))
    din("onesW", (max(h, w), 1), BF16)
    din("ones1", (1, 64), BF16)
    y_cols = nc_cols if tiny_out else hw
    y_d = nc.dram_tensor("y", (bpc, DIM, y_cols), BF16,
                         kind="ExternalOutput").ap()

    with tile.TileContext(nc) as tc:
        _emit(nc, tc, dram, y_d, bpc, h, w, hw, chunk_h, n_chunks, nc_cols,
              nrep, tiny_out)
    nc.compile()
    return nc


def _emit(nc, tc, dram, y_d, bpc, h, w, hw, chunk_h, n_chunks, nc_cols,
          nrep=1, tiny_out=False):
    from contextlib import ExitStack
    with ExitStack() as _ctx:
        _emit_body(_ctx, nc, tc, dram, y_d, bpc, h, w, hw, chunk_h, n_chunks,
                   nc_cols, nrep, tiny_out)


def _emit_body(ctx, nc, tc, dram, y_d, bpc, h, w, hw, chunk_h, n_chunks,
               nc_cols, nrep=1, tiny_out=False):
    ctx.enter_context(nc.allow_low_precision(
        reason="bf16/fp8 matmul operand rounding"))
    # ---- persistent weights / params (loaded once) ----
    wp = ctx.enter_context(tc.tile_pool(name="weights", bufs=1))

    ident128 = wp.tile([128, 128], MMDT, tag="ident", name="ident128")
    nc.scalar.dma_start(out=ident128, in_=dram["ident"])
    wbig = wp.tile([128, WBIG_COLS], MMDT, tag="wbig")
    nc.scalar.dma_start(out=wbig, in_=dram["wbig"])
    wbigb = wp.tile([128, WBIGB_COLS], BF16, tag="wbigb")
    nc.scalar.dma_start(out=wbigb, in_=dram["wbigb"])
    # DoubleRow stationary tiles must be native [p, 2, m] 3-D tiles
    wp8t = []
    for mo in range(2):
        t8 = wp.tile([128, 2, 128], FP8, tag=f"wp8_{mo}", name=f"wp8_{mo}")
        nc.scalar.dma_start(out=t8.rearrange("p i f -> p (i f)"),
                          in_=dram["wp8"][:, 256 * mo:256 * (mo + 1)])
        wp8t.append(t8)
    params = wp.tile([128, 20], F32, tag="params")
    nc.scalar.dma_start(out=params, in_=dram["params"])
    qkb = wp.tile([128, 512], F32, tag="qkb")
    nc.scalar.dma_start(out=qkb, in_=dram["qkbias"])

    def _slices(tile_, widths):
        out, off = [], 0
        for wd in widths:
            out.append(tile_[:, off:off + wd])
            off += wd
        return out

    (wqT0, wqT1, wkT0, wkT1, wvT0, wvT1, wpw0, wpw1, wpw2, wpw3) = _slices(
        wbig, [NH_KD, NH_KD, NH_KD, NH_KD, DH, DH,
               DIM, DIM, DIM, DIM])
    wqT, wkT, wvT = [wqT0, wqT1], [wkT0, wkT1], [wvT0, wvT1]
    wpwT = [wpw0, wpw1, wpw2, wpw3]
    (wrT0, wrT1, wcT0, wcT1, wqTp0, wqTp1, wkTp0, wkTp1, wvTb0, wvTb1,
     ident64b) = _slices(
        wbigb, [DH, DH, DH, DH, 256, 256, 256, 256, DH, DH, 64])
    wrT, wcT = [wrT0, wrT1], [wcT0, wcT1]
    wqTp, wkTp, wvTb = [wqTp0, wqTp1], [wkTp0, wkTp1], [wvTb0, wvTb1]
    ident64 = ident64b[:64, :]
    onesW = wp.tile([max(h, w), 1], BF16, tag="onesW")   # value = W (mean fold)
    nc.scalar.dma_start(out=onesW, in_=dram["onesW"])
    ones1 = wp.tile([1, 64], BF16, tag="ones1")
    nc.scalar.dma_start(out=ones1, in_=dram["ones1"])

    # param columns
    zsv = [params[:, g:g + 1] for g in range(2)]          # zscale for v grps
    zbias = [params[:, 2 + g:3 + g] for g in range(4)]    # q, k, v0, v1
    bv_att = [params[:, 6 + m:7 + m] for m in range(2)]
    brv = [params[:, 8 + m:9 + m] for m in range(2)]
    bcc = [params[:, 10 + m:11 + m] for m in range(2)]
    bp3 = [params[:, 12 + m:13 + m] for m in range(2)]
    bpw6 = [params[:, 14 + m:15 + m] for m in range(2)]

    # ---- pools ----
    px = ctx.enter_context(tc.tile_pool(name="x", bufs=2))
    pa = ctx.enter_context(tc.tile_pool(name="attn", bufs=1))
    pz = ctx.enter_context(tc.tile_pool(name="z", bufs=5))
    pc = ctx.enter_context(tc.tile_pool(name="chunk", bufs=3))
    pout = ctx.enter_context(tc.tile_pool(name="outb", bufs=3))
    pp = ctx.enter_context(tc.tile_pool(name="psum", bufs=1, space="PSUM"))

    def phase_load(b):
        xs = []
        for k in range(2):
            t = px.tile([128, hw], MMDT, tag=f"xs{k}")
            for c in range(n_chunks):
                cs0 = slice(c * nc_cols, (c + 1) * nc_cols)
                nc.sync.dma_start(out=t[:, cs0],
                                  in_=dram["x"][b, 128 * k:128 * (k + 1), cs0])
            xs.append(t)
        return xs

    def means_alloc():
        return [pa.tile([128, 128], BF16, tag=f"xm{k}", bufs=4,
                        name=f"xm{k}") for k in range(2)]

    def means_piece(xs, xm, piece):
        # One of 4 pieces of the row/col sums of x: (dir, k). Identity-matmul
        # accumulation on PE (psum on the "att" tag so it never serializes
        # against the chunk psum rings), small reduce on DVE. xm[k] layout:
        # [128, 128] bf16, cols 0..63 = W-dir sums, 64..127 = H-dir sums.
        wt = max(w // 8, 1)
        wgroups = w // wt
        dirw, k = piece // 2, piece % 2
        ptag = "att" if piece % 2 == 0 else "p0"
        if dirw == 0:
            psw = pp.tile([128, h * wt], F32, tag=ptag, name=f"ps_mw{k}")
            xv = xs[k].rearrange("p (h j t) -> p j h t", j=wgroups, t=wt)
            for j in range(wgroups):
                nc.tensor.matmul(psw, lhsT=ident128, rhs=xv[:, j],
                                 start=(j == 0), stop=(j == wgroups - 1))
            with tc.high_priority(offset=64):
                nc.vector.tensor_reduce(
                    out=xm[k][:, 0:h].unsqueeze(-1),
                    in_=psw.rearrange("p (h t) -> p h t", t=wt),
                    axis=AX.X, op=ALU.add)
        else:
            psh = pp.tile([128, nc_cols], F32, tag=ptag, name=f"ps_mh{k}")
            for c in range(n_chunks):
                nc.tensor.matmul(
                    psh, lhsT=ident128,
                    rhs=xs[k][:, c * nc_cols:(c + 1) * nc_cols],
                    start=(c == 0), stop=(c == n_chunks - 1))
            with tc.high_priority(offset=64):
                nc.vector.tensor_reduce(
                    out=xm[k][:, h:h + w].unsqueeze(-1),
                    in_=psh.rearrange("p (s w) -> p w s", w=w),
                    axis=AX.X, op=ALU.add)

    def phase_means(xs):
        xm = means_alloc()
        for piece in range(4):
            means_piece(xs, xm, piece)
        return xm

    def phase_attn(xm):
        """Both axial attentions. Returns (xr_f, xc_f): [2](128, nseq) bf16
        conv outputs + bias, pre-broadcast."""
        # q/k projections for BOTH directions at once:
        # qk psum [128, 512] = [q_t0 | q_t1 | k_t0 | k_t1], each 128 cols =
        # [dir-row 64 | dir-col 64]; padded head layout (16 kd + 16 zero).
        qk_ps = pp.tile([128, 512], F32, tag="att", name="qk_ps")
        for wi, wT in enumerate((wqTp, wkTp)):
            for t in range(2):
                sl = slice(256 * wi + 128 * t, 256 * wi + 128 * (t + 1))
                for k in range(2):
                    nc.tensor.matmul(qk_ps[:, sl],
                                     lhsT=wT[k][:, 128 * t:128 * (t + 1)],
                                     rhs=xm[k], start=(k == 0), stop=(k == 1))
        qk_sb = pa.tile([128, 512], BF16, tag="qk_sb")
        nc.vector.tensor_tensor(out=qk_sb, in0=qk_ps, in1=qkb, op=ALU.add)

        xatt = [None, None]
        for d in range(2):          # 0 = row (nseq=h), 1 = col (nseq=w)
            nseq = h if d == 0 else w
            do = 64 * d
            # scoresT [j, (h i)]. Heads h=j and h=j+4 share the partition
            # strip 32j; they go into one per-j psum tile (mixing different
            # tile_position rows inside one psum tile crashes the exec unit).
            eT = pa.tile([nseq, HEADS * nseq], BF16, tag=f"at_e{d}")
            eTv = eT.rearrange("p (t j i) -> p j t i", t=2, j=4)
            for j in range(4):
                st_ps = pp.tile([nseq, 2 * nseq], F32, tag="att",
                                name=f"st_ps{d}{j}")
                for t in range(2):
                    ksl = qk_sb[32 * j:32 * (j + 1),
                                256 + 128 * t + do:256 + 128 * t + do + 64]
                    qsl = qk_sb[32 * j:32 * (j + 1),
                                128 * t + do:128 * t + do + 64]
                    nc.tensor.matmul(st_ps[:, nseq * t:nseq * (t + 1)],
                                     lhsT=ksl, rhs=qsl, start=True, stop=True,
                                     tile_position=(32 * j, 0))
                nc.scalar.activation(
                    out=eTv[:, j], in_=st_ps.rearrange("p (t i) -> p t i", t=2),
                    func=AF.Exp, scale=KEY_DIM ** -0.5)
            # denominator row (scaled by W via onesW value) and reciprocal
            srow_ps = pp.tile([1, HEADS * nseq], F32, tag="att",
                              name=f"srow{d}")
            nc.tensor.matmul(srow_ps, lhsT=onesW[:nseq, :], rhs=eT,
                             start=True, stop=True)
            recip = pa.tile([1, HEADS * nseq], BF16, tag=f"at_rc{d}")
            nc.vector.reciprocal(out=recip, in_=srow_ps)
            rb_ps = pp.tile([nseq, HEADS * nseq], F32, tag="att",
                            name=f"rb{d}")
            nc.tensor.matmul(rb_ps, lhsT=ones1[:, :nseq], rhs=recip,
                             start=True, stop=True)
            eTn = pa.tile([nseq, HEADS * nseq], BF16, tag=f"at_en{d}")
            nc.vector.tensor_tensor(out=eTn, in0=eT, in1=rb_ps, op=ALU.mult)
            # vrT (nseq, 256) = xm_dir.T @ Wv.T
            vrT_ps = pp.tile([nseq, DH], F32, tag="att", name=f"vrT{d}")
            for k in range(2):
                nc.tensor.matmul(vrT_ps, lhsT=xm[k][:, do:do + nseq],
                                 rhs=wvTb[k], start=(k == 0), stop=(k == 1))
            vrT = pa.tile([nseq, DH], BF16, tag=f"at_vs{d}")
            nc.vector.tensor_copy(out=vrT, in_=vrT_ps)
            # attention out, transposed: xrT[i, 32h+d]
            xrT_ps = pp.tile([nseq, DH], F32, tag="att", name=f"xrT{d}")
            for hh in range(HEADS):
                nc.tensor.matmul(xrT_ps[:, 32 * hh:32 * (hh + 1)],
                                 lhsT=eTn[:, nseq * hh:nseq * (hh + 1)],
                                 rhs=vrT[:, 32 * hh:32 * (hh + 1)],
                                 start=True, stop=True)
            xrT_sb = pa.tile([nseq, DH], BF16, tag=f"at_xt{d}")
            nc.vector.tensor_copy(out=xrT_sb, in_=xrT_ps)
            # transpose back to (channel, i), relu(+bv) on eviction
            xr_relu = []
            for t in range(2):
                tr_ps = pp.tile([128, nseq], BF16, tag="att",
                                name=f"at_tr{d}_{t}")
                nc.tensor.transpose(tr_ps, xrT_sb[:, 128 * t:128 * (t + 1)],
                                    ident64[:nseq, :nseq])
                sb = pa.tile([128, nseq], BF16, tag=f"at_xrr{d}_{t}")
                nc.scalar.activation(out=sb, in_=tr_ps, func=AF.Relu,
                                     bias=bv_att[t])
                xr_relu.append(sb)
            # conv (dh -> dh) + bias
            wconvT = wrT if d == 0 else wcT
            bconv = brv if d == 0 else bcc
            xa = []
            for m in range(2):
                ps = pp.tile([128, nseq], F32, tag="att", name=f"at_cv{d}{m}")
                for k in range(2):
                    nc.tensor.matmul(ps,
                                     lhsT=wconvT[k][:, 128 * m:128 * (m + 1)],
                                     rhs=xr_relu[k],
                                     start=(k == 0), stop=(k == 1))
                sb = pa.tile([128, nseq], BF16, tag=f"at_xa{d}_{m}")
                nc.scalar.activation(out=sb, in_=ps, func=AF.Identity,
                                     bias=bconv[m])
                xa.append(sb)
            xatt[d] = xa
        return xatt[0], xatt[1]

    def phase_chunks(b, xs, xr_f, xc_f, c_lo=0, c_hi=None):
        for c in range(c_lo, c_hi if c_hi is not None else n_chunks):
            cs = slice(c * nc_cols, (c + 1) * nc_cols)
            hs = slice(c * chunk_h, (c + 1) * chunk_h)
            # q/k/v matmuls (q,k weights carry the depthwise gate scale)
            grp_ps = []
            for gi, (wT, mo) in enumerate(((wqT, 0), (wkT, 0),
                                           (wvT, 0), (wvT, 1))):
                ps = pp.tile([128, nc_cols], F32, tag=f"mm{gi}")
                for k in range(2):
                    nc.tensor.matmul(
                        ps, lhsT=wT[k][:, 128 * mo:128 * (mo + 1)],
                        rhs=xs[k][:, cs], start=(k == 0), stop=(k == 1))
                grp_ps.append(ps)
            # z evictions -> bf16 (q/k: relu+bias; v: relu+scale+bias)
            z = []
            for g in range(4):
                sb = pz.tile([128, nc_cols], MMDT, tag=f"z{g}")
                if g < 2:
                    nc.scalar.activation(out=sb, in_=grp_ps[g], func=AF.Relu,
                                         bias=zbias[g])
                else:
                    nc.scalar.activation(out=sb, in_=grp_ps[g], func=AF.Relu,
                                         scale=zsv[g - 2], bias=zbias[g])
                z.append(sb)
            # xx = relu(v + xr + xc) -> fp8 (i-major halves for DoubleRow)
            xx8 = pc.tile([128, 2, 512], FP8, tag="xx8")
            for m in range(2):
                rc = pc.tile([128, chunk_h, w], BF16, tag=f"rc{m}")
                nc.gpsimd.tensor_tensor(
                    out=rc,
                    in0=xr_f[m][:, hs].unsqueeze(-1).broadcast_to(
                        (128, chunk_h, w)),
                    in1=xc_f[m].unsqueeze(1).broadcast_to((128, chunk_h, w)),
                    op=ALU.add)
                t = pc.tile([128, nc_cols], BF16, tag=f"xx{m}")
                with tc.high_priority(offset=64):
                    nc.vector.scalar_tensor_tensor(
                        out=t, in0=grp_ps[2 + m], scalar=0.0,
                        in1=rc.rearrange("p h w -> p (h w)"),
                        op0=ALU.add, op1=ALU.add)
                    nc.vector.tensor_scalar(
                        out=xx8[:, m, :], in0=t,
                        scalar1=0.0, scalar2=0.0, op0=ALU.max, op1=ALU.add)
            # pointwise conv (512 -> 256) in bf16; bias folded into gate stt
            qkv_ps = []
            for m in range(2):
                ps = pp.tile([128, nc_cols], F32, tag=f"o{m}")
                for k in range(4):
                    nc.tensor.matmul(
                        ps, lhsT=wpwT[k][:, 128 * m:128 * (m + 1)],
                        rhs=z[k], start=(k == 0), stop=(k == 3))
                qkv_ps.append(ps)
            # final conv (256 -> 256) in fp8 DoubleRow; relu+bias on ACT;
            # gate stt on DVE
            for m in range(2):
                ps = pp.tile([128, nc_cols], F32, tag="p0", name=f"xp{m}")
                if USE_DR:
                    nc.tensor.matmul(ps, lhsT=wp8t[m], rhs=xx8,
                                     start=True, stop=True, perf_mode=DR)
                else:
                    for i in range(2):
                        nc.tensor.matmul(ps, lhsT=wp8t[m][:, i], rhs=xx8[:, i],
                                         start=(i == 0), stop=(i == 1))
                r = pc.tile([128, nc_cols], BF16, tag=f"r{m}")
                nc.scalar.activation(out=r, in_=ps, func=AF.Relu,
                                     scale=1.0 / 64.0, bias=bp3[m])
                r6 = pc.tile([128, nc_cols], BF16, tag=f"r6{m}")
                nc.vector.tensor_scalar(out=r6, in0=r, scalar1=6.0,
                                        scalar2=0.0, op0=ALU.min, op1=ALU.add)
                o = pout.tile([128, nc_cols], BF16, tag=f"ob{m}")
                nc.vector.scalar_tensor_tensor(
                    out=o, in0=qkv_ps[m], scalar=bpw6[m], in1=r6,
                    op0=ALU.add, op1=ALU.mult)
                ocs = slice(0, nc_cols) if tiny_out else cs
                nc.sync.dma_start(out=y_d[b, 128 * m:128 * (m + 1), ocs],
                                  in_=o)

    for _ in range(nrep):
        # Emission order = engine FIFO + tag-grant order. Chunks lead (they
        # only need x); means for the NEXT batch are spread as PE filler
        # between the previous batch's first-half chunks (their psums live on
        # the "att" tag so they never serialize against the chunk rings),
        # then attention runs before the second half.
        held = None
        half = n_chunks // 2
        for b in range(bpc):
            xs = phase_load(b)
            if held is not None:
                phase_chunks(*held, c_lo=0, c_hi=half)
            xm = phase_means(xs)
            at = phase_attn(xm)
            if held is not None:
                phase_chunks(*held, c_lo=half)
            held = (b, xs, *at)
        phase_chunks(*held)


# ---------------------------------------------------------------------------
# host-side preparation
# ---------------------------------------------------------------------------

def _interp_pos_np(pe, n):
    s = pe.shape[-1]
    pos = np.clip((np.arange(n, dtype=np.float64) + 0.5) * (s / n) - 0.5,
                  0.0, s - 1.0).astype(np.float32)
    i0 = np.floor(pos).astype(np.int32)
    i1 = np.minimum(i0 + 1, s - 1)
    fw = (pos - i0).astype(np.float32)
    return pe[:, i0] * (1.0 - fw) + pe[:, i1] * fw


def prepare_consts(inputs, h=H, w=W, chunk_h=8):
    """Fold BN/scales and build the constant tensors the kernel expects."""
    import ml_dtypes
    f = lambda a: np.ascontiguousarray(np.asarray(a, dtype=np.float32))
    fb = lambda a: np.ascontiguousarray(
        np.asarray(a, dtype=np.float32).astype(ml_dtypes.bfloat16))
    Wq, sq, bq = f(inputs["Wq"]), f(inputs["sq"]), f(inputs["bq"])
    Wk, sk, bk = f(inputs["Wk"]), f(inputs["sk"]), f(inputs["bk"])
    Wv, sv, bv = f(inputs["Wv"]), f(inputs["sv"]), f(inputs["bv"])
    wdw, sdw, bdw = f(inputs["wdw"]), f(inputs["sdw"]), f(inputs["bdw"])
    Wpw, spw, bpw = f(inputs["Wpw"]), f(inputs["spw"]), f(inputs["bpw"])
    Wr, sr, br = f(inputs["Wr"]), f(inputs["sr"]), f(inputs["br"])
    Wc, sc, bc = f(inputs["Wc"]), f(inputs["sc"]), f(inputs["bc"])
    Wp, sp, bp = f(inputs["Wp"]), f(inputs["sp"]), f(inputs["bp"])

    Wq_f = sq[:, None] * Wq
    Wk_f = sk[:, None] * Wk
    Wv_f = sv[:, None] * Wv

    g = wdw * sdw
    bqkv = np.concatenate([bq, bk, bv])
    zbias = g * bqkv + bdw          # 512
    g_q, g_k, g_v = g[:NH_KD], g[NH_KD:2 * NH_KD], g[2 * NH_KD:]

    def tiles2(a):   # (256, cols) -> [(128, cols)] * 2
        return [a[:128], a[128:]]

    # q/k conv weights carry the depthwise gate scale
    wbig = np.concatenate(
        tiles2((g_q[None, :] * Wq_f.T)) + tiles2((g_k[None, :] * Wk_f.T))
        + tiles2(Wv_f.T)
        + [(((spw[:, None] * Wpw) / 6.0).T)[128 * k:128 * (k + 1)]
           for k in range(4)], axis=1)
    consts = {"wbig": f(wbig), "ident": np.eye(128, dtype=np.float32)}
    # padded head layout for the attention q/k weights (1/mean fold included)
    assert h == w, "mean folds assume H == W"
    wqTp = np.zeros((DIM, 256), np.float32)
    wkTp = np.zeros((DIM, 256), np.float32)
    # qk bias layout matches the qk psum: [q_t0 | q_t1 | k_t0 | k_t1],
    # 128 cols each = [row-dir 64 | col-dir 64]; partitions 32h'+kd padded.
    qk_b = np.zeros((128, 512), np.float32)
    pe_rq = _interp_pos_np(f(inputs["pe_rq"]), h)
    pe_rk = _interp_pos_np(f(inputs["pe_rk"]), h)
    pe_cq = _interp_pos_np(f(inputs["pe_cq"]), w)
    pe_ck = _interp_pos_np(f(inputs["pe_ck"]), w)
    for hh in range(HEADS):
        sl_p = slice(32 * hh, 32 * hh + KEY_DIM)
        sl_c = slice(KEY_DIM * hh, KEY_DIM * (hh + 1))
        wqTp[:, sl_p] = (Wq_f[sl_c, :] / w).T
        wkTp[:, sl_p] = (Wk_f[sl_c, :] / w).T
        t, j = hh // 4, hh % 4
        prow = slice(32 * j, 32 * j + KEY_DIM)
        qk_b[prow, 128 * t:128 * t + 64] = bq[sl_c, None] + pe_rq[sl_c, :]
        qk_b[prow, 128 * t + 64:128 * t + 128] = bq[sl_c, None] + pe_cq[sl_c, :]
        qk_b[prow, 256 + 128 * t:256 + 128 * t + 64] = (
            bk[sl_c, None] + pe_rk[sl_c, :])
        qk_b[prow, 256 + 128 * t + 64:256 + 128 * t + 128] = (
            bk[sl_c, None] + pe_ck[sl_c, :])
    consts["qkbias"] = f(qk_b)
    id64pad = np.zeros((128, 64), np.float32)
    id64pad[:64] = np.eye(64, dtype=np.float32)
    wbigb = np.concatenate(
        tiles2((sr[:, None] * Wr).T) + tiles2((sc[:, None] * Wc).T)
        + tiles2(wqTp) + tiles2(wkTp) + tiles2(Wv_f.T)
        + [id64pad], axis=1)
    consts["wbigb"] = fb(wbigb)
    # fp8 DoubleRow final-conv weights: wp8[p, i*256 + m] = 64*Wp_f[m, p+128i]
    Wp_f = sp[:, None] * Wp
    wp8 = np.zeros((128, 512), np.float32)
    for mo in range(2):
        for i in range(2):
            wp8[:, 256 * mo + 128 * i:256 * mo + 128 * (i + 1)] = (
                64.0 * Wp_f[128 * mo:128 * (mo + 1),
                            128 * i:128 * (i + 1)].T)
    consts["wp8"] = np.ascontiguousarray(wp8.astype(ml_dtypes.float8_e4m3))

    params = np.zeros((128, 20), np.float32)
    params[:, 0:2] = g_v.reshape(2, 128).T
    params[:, 2:6] = zbias.reshape(4, 128).T
    params[:, 6:8] = bv.reshape(2, 128).T
    params[:, 8:10] = (br + bv).reshape(2, 128).T
    params[:, 10:12] = bc.reshape(2, 128).T
    params[:, 12:14] = (bp + 3.0).reshape(2, 128).T
    params[:, 14:16] = (bpw / 6.0).reshape(2, 128).T
    consts["params"] = f(params)
    consts["onesW"] = np.full((max(h, w), 1), float(w),
                              ml_dtypes.bfloat16)
    consts["ones1"] = np.ones((1, 64), ml_dtypes.bfloat16)
    return consts


_NC_CACHE = {}


def _get_nc():
    if "nc" not in _NC_CACHE:
        _NC_CACHE["nc"] = build_nc()
    return _NC_CACHE["nc"]


def kernel(**inputs) -> np.ndarray:
    x = np.ascontiguousarray(np.asarray(inputs["x"], dtype=np.float32))
    consts = prepare_consts(inputs)
    nc = _get_nc()
    in_maps = []
    for c in range(N_CORES):
        m = dict(consts)
        m["x"] = np.ascontiguousarray(
            x[c * BPC:(c + 1) * BPC].reshape(BPC, DIM, H * W))
        in_maps.append(m)
    res = bass_utils.run_bass_kernel_spmd(nc, in_maps,
                                          core_ids=list(range(N_CORES)))
    y = np.concatenate([np.asarray(r["y"], dtype=np.float32)
                        for r in res.results], axis=0)
    return y.reshape(B, DIM, H, W)
